# revision 6
# baseline (speedup 1.0000x reference)
"""Trainium2 kernel for nn_KernelizedAttention_14869176779022.

Math note: the reference computes
    out = (s * v) / s        with s = <phi_q, phi_k> > 0  (sums of exps)
so out == v == x @ Wv.T + bv exactly (up to one multiply/divide rounding).
The kernel therefore only computes the Wv linear layer.

Sharding: data-parallel over the 8192 (B*S) positions - 1024 rows per core.
Wv (pre-transposed, x64-scaled) is replicated; the x64 unscale and the +bv
bias ride the host-side unshard pass (which already upcasts bf16 -> f32).

v3 design (from v2 trace analysis):
  - All inputs e3m4; the PE consumes fp8 directly in normal mode (bf16 rate,
    numerically identical to a cast path since e3m4 -> bf16 is exact).
  - Measured queue behavior: each HWDGE ring's FIRST chunk runs fast
    (~105-170GB/s) but degrades to ~37-40GB/s once SWDGE traffic starts;
    SWDGE ramps ~110 -> 250-360GB/s. So the two HWDGE rings carry exactly
    one chunk each (wv A-half k0-3 / k4-7, landing ~10us) and SWDGE carries
    x m0..m7 in consumption order, then wv B-half, then the output stores.
  - x is fed per m-tile (one semaphore covers all 8 k-slices), so after an
    m-block's first matmul fires the rest never stall.
  - Any multi-us PE idle hole resets the HAM clock-gate's 3.4us busy window
    (v2 warmed at 16.3us instead of 11.3); dummies bridge preamble-end to
    the first x chunk with a fine N=128 tail so the window stays clean.
  - Output stores go per full m-tile (256KB, 2KB DRAM lines) on SWDGE right
    after each B-drain; out DRAM is per-partition packed ([P, MT*E]) and the
    host unpermutes. HWDGE-ring stores measured only ~37GB/s - avoid them.
"""

import sys

if "/opt/trn_rl_repo" not in sys.path:
    sys.path.insert(0, "/opt/trn_rl_repo")

import numpy as np

B, S, E = 2, 4096, 1024
N_CORES = 8
ROWS = B * S            # 8192
R = ROWS // N_CORES     # 1024 rows per core
P = 128                 # partitions
KT = E // P             # 8 contraction tiles
MT = R // P             # 8 row tiles per core
NH = 2                  # n-half passes (512 output cols each)
NSZ = E // NH           # 512 = one PSUM bank (fp32)

_NC_CACHE = {}


def _build_nc(**bass_kwargs):
    import concourse.bass as bass
    import concourse.mybir as mybir
    from concourse import bacc
    from concourse.tile import TileContext

    f32 = mybir.dt.float32
    bf16 = mybir.dt.bfloat16
    fp8 = mybir.dt.float8e3
    nc = bacc.Bacc(None, target_bir_lowering=False, **bass_kwargs)

    # xb[p, (m*KT + k)*P + mm] = x_shard[m*P + mm, k*P + p]  (e3m4)
    xb = nc.dram_tensor("xb", [P, MT * KT * P], fp8, kind="ExternalInput")
    # wv[p, (h*KT + k)*NSZ + c] = 64*Wv[h*NSZ + c, k*P + p]  (e3m4; x64 keeps
    # Wv ~N(0,1/32) in e3m4's normal range; undone on host)
    wv = nc.dram_tensor("wv", [P, NH * KT * NSZ], fp8, kind="ExternalInput")
    # per-partition packed output: outp[p, m*E + c] = out_row[m*P + p, c]
    # (host unpermutes); full-tile stores get 2KB DRAM lines this way
    out = nc.dram_tensor("out", [P, MT * E], bf16, kind="ExternalOutput")

    xm = KT * P             # one m-tile of x: 1024 cols = 128KB e3m4

    with TileContext(nc) as tc:
        with (
            tc.tile_pool(name="consts", bufs=1) as consts,
            tc.tile_pool(name="xpool", bufs=1) as xpool,
            tc.tile_pool(name="wpool", bufs=1) as wpool,
            tc.tile_pool(name="opool", bufs=MT) as opool,
            tc.tile_pool(name="ppool", bufs=7, space="PSUM") as ppool,
            tc.tile_pool(name="dpool", bufs=1, space="PSUM") as dpool,
        ):
            # PE warm-up: dummy matmuls keep the PE busy from preamble end
            # (~7.9us) to the first x chunk (~10.5us): 5 N=512 (427ns cold)
            # + 4 N=128 (107ns) for a fine-grained bridge. Memset runs on
            # the vector engine so gpsimd can issue SWDGE immediately.
            dum_sb = consts.tile([P, NSZ], bf16, tag="dum")
            nc.vector.memset(dum_sb, 0.0)
            dum_ps = dpool.tile([P, NSZ], f32, tag="dps")
            for _ in range(7):
                nc.tensor.matmul(
                    dum_ps, dum_sb[:, :P], dum_sb, start=True, stop=True
                )
            for _ in range(2):
                nc.tensor.matmul(
                    dum_ps[:, :P], dum_sb[:, :P], dum_sb[:, :P],
                    start=True, stop=True,
                )

            wv_sb = wpool.tile([P, NH * KT * NSZ], fp8, tag="wv")
            x_sb = xpool.tile([P, MT * KT * P], fp8, tag="x")

            # Input stream. 256KB chunks keep 2KB DRAM lines (~125GB/s on a
            # HWDGE ring even under contention; 1KB-line chunks crawl at
            # ~40GB/s after a ring's first chunk). SWDGE chunk semaphores
            # fire late when followers interleave, so the two head chunks
            # (x m0m1, wv-A k0-3) ride the HWDGE rings.
            nc.sync.dma_start(out=x_sb[:, 0 : 2 * xm], in_=xb[:, 0 : 2 * xm])
            nc.scalar.dma_start(out=wv_sb[:, 0 : 4 * NSZ], in_=wv[:, 0 : 4 * NSZ])
            # SWDGE: wv-A k4-7, then x m2..m7 in pairs, then wv-B.
            nc.gpsimd.dma_start(
                out=wv_sb[:, 4 * NSZ : 8 * NSZ], in_=wv[:, 4 * NSZ : 8 * NSZ]
            )
            for mp in range(1, MT // 2):
                nc.gpsimd.dma_start(
                    out=x_sb[:, 2 * mp * xm : 2 * (mp + 1) * xm],
                    in_=xb[:, 2 * mp * xm : 2 * (mp + 1) * xm],
                )
            nc.gpsimd.dma_start(
                out=wv_sb[:, 8 * NSZ : 12 * NSZ], in_=wv[:, 8 * NSZ : 12 * NSZ]
            )
            nc.gpsimd.dma_start(
                out=wv_sb[:, 12 * NSZ : 16 * NSZ], in_=wv[:, 12 * NSZ : 16 * NSZ]
            )

            om_tiles = [
                opool.tile([P, E], bf16, name=f"om{m}", tag="om")
                for m in range(MT)
            ]

            def store_half(m, h, ring):
                dst = bass.AP(
                    tensor=out.tensor if hasattr(out, "tensor") else out,
                    offset=m * E + h * NSZ,
                    ap=[[MT * E, P], [1, NSZ]],
                )
                ring.dma_start(
                    out=dst, in_=om_tiles[m][:, h * NSZ : (h + 1) * NSZ]
                )

            def drain(h, m, ps):
                nc.vector.tensor_copy(
                    out=om_tiles[m][:, h * NSZ : (h + 1) * NSZ], in_=ps
                )
                # A-halves ride the otherwise-idle HWDGE rings (slow but
                # plenty of slack); B-halves go on SWDGE right after their
                # drain so the final store is short.
                if h == 0:
                    store_half(m, 0, nc.sync if (m % 2 == 0) else nc.scalar)
                else:
                    store_half(m, 1, nc.gpsimd)

            def fillers(n):
                for _ in range(n):
                    nc.tensor.matmul(
                        dum_ps[:, :P], dum_sb[:, :P], dum_sb[:, :P],
                        start=True, stop=True,
                    )

            def mblock(m, h, ps):
                for k in range(KT):
                    nc.tensor.matmul(
                        ps,
                        x_sb[:, (m * KT + k) * P : (m * KT + k + 1) * P],
                        wv_sb[:, (h * KT + k) * NSZ : (h * KT + k + 1) * NSZ],
                        start=(k == 0),
                        stop=(k == KT - 1),
                    )
                    if h == 0 and m == 0 and k == 3:
                        fillers(2)

            # A-pass (h=0) then B-pass (h=1), m-outer; x per-m-pair semaphores
            # and early-resident wv keep blocks stall-free after their first MM.
            for h in range(NH):
                for m in range(MT):
                    ps = ppool.tile([P, NSZ], f32, name=f"ps{h}{m}", tag="ps")
                    mblock(m, h, ps)
                    drain(h, m, ps)
    nc.compile()
    return nc


def _get_nc():
    if "nc" not in _NC_CACHE:
        _NC_CACHE["nc"] = _build_nc()
    return _NC_CACHE["nc"]


def _prep_in_maps(x, Wv):
    import ml_dtypes

    e3m4 = ml_dtypes.float8_e3m4
    x = np.ascontiguousarray(np.asarray(x, dtype=np.float32))
    Wv = np.asarray(Wv, dtype=np.float32)

    xf = x.reshape(ROWS, E)
    # wvp[p, (h*KT + k)*NSZ + c] = 64*Wv[h*NSZ + c, k*P + p]
    wvp = np.ascontiguousarray(
        (Wv * 64.0)
        .reshape(NH, NSZ, KT, P)
        .transpose(3, 0, 2, 1)
        .reshape(P, NH * KT * NSZ)
        .astype(e3m4)
    )

    in_maps = []
    for c in range(N_CORES):
        xs = xf[c * R : (c + 1) * R]                    # [R, E]
        # xb[p, (m*KT+k)*P+mm] = xs[m*P+mm, k*P+p]
        xbc = np.ascontiguousarray(
            xs.reshape(MT, P, KT, P)
            .transpose(3, 0, 2, 1)
            .reshape(P, MT * KT * P)
            .astype(e3m4)
        )
        in_maps.append({"xb": xbc, "wv": wvp})
    return in_maps


def _install_ntff_hook():
    """This image's antenv lacks axon_hooks; recreate the bridge module so
    run_bass_kernel_spmd(trace=True) can reach the ctypes NTFF profiler."""
    import types

    if "antenv.axon_hooks" in sys.modules:
        return
    try:
        from trn_agent_boot.trn_boot import _ntff_profile_via_ctypes
    except ImportError:
        return
    hook = _ntff_profile_via_ctypes("/opt/axon/libaxon_pjrt.so")
    mod = types.ModuleType("antenv.axon_hooks")
    mod._hook = hook
    mod.get_axon_ntff_profile_hook = lambda: mod._hook
    mod.set_axon_ntff_profile_hook = lambda h: setattr(mod, "_hook", h)
    sys.modules["antenv.axon_hooks"] = mod


def _run(x, Wv, bv, trace=False):
    from concourse.bass_utils import run_bass_kernel_spmd

    if trace:
        _install_ntff_hook()
    nc = _get_nc()
    in_maps = _prep_in_maps(x, Wv)
    res = run_bass_kernel_spmd(
        nc, in_maps, core_ids=list(range(N_CORES)), trace=trace
    )
    # outp[p, m*E + c] = out_row[m*P + p, c]  ->  [R, E]
    shards = []
    for c in range(N_CORES):
        o = np.asarray(res.results[c]["out"])            # [P, MT*E]
        shards.append(
            o.reshape(P, MT, E).transpose(1, 0, 2).reshape(R, E)
        )
    out = np.concatenate(shards, axis=0)
    out = out.reshape(B, S, E).astype(np.float32) * (1.0 / 64.0)
    out += np.asarray(bv, dtype=np.float32)
    return out, res


def kernel(x, Wq, bq, Wk, bk, Wv, bv, weights):
    out, _ = _run(x, Wv, bv, trace=False)
    return out


def kernel_traced(x, Wq, bq, Wk, bk, Wv, bv, weights):
    """Like kernel() but with NTFF profiling; returns (out, BassKernelResults)."""
    out, res = _run(x, Wv, bv, trace=True)
    return out, res


# revision 9
# speedup vs baseline: 1.0544x; 1.0544x over previous
"""Trainium2 kernel for nn_KernelizedAttention_14869176779022.

Math note: the reference computes
    out = (s * v) / s        with s = <phi_q, phi_k> > 0  (sums of exps)
so out == v == x @ Wv.T + bv exactly (up to one multiply/divide rounding).
The kernel therefore only computes the Wv linear layer.

Sharding: data-parallel over the 8192 (B*S) positions - 1024 rows per core.
Wv (pre-transposed, x64-scaled) is replicated; the x64 unscale and the +bv
bias ride the host-side unshard pass (which already upcasts bf16 -> f32).

v5 design (measured-queue-model driven):
  - All inputs e3m4; PE consumes fp8 directly in normal mode (bf16 rate,
    numerically identical to a cast path since e3m4 -> bf16 is exact).
  - Inputs are packed host-side into ONE DRAM tensor in exact consumption
    order, so each DMA chunk is a contiguous block with 2-5KB partition
    lines (HWDGE rings crawl at ~40GB/s on 1KB lines vs ~125+ on 2KB+).
  - The sync ring (lowest latency, most predictable) carries the critical
    head: [wvA k0-3 | x m0] then [wvA k4-7]. Scalar carries [x m1 m2].
    SWDGE (whose chunk semaphores fire late while followers interleave)
    carries only slack-tolerant bulk: [x m3 m4], [x m5 m6], [x m7 | wvB].
  - Dummy matmuls bridge preamble-end (~7.9us) to the head chunk (~11us)
    with no idle hole (any multi-us hole resets the HAM clock-gate's 3.4us
    busy window and everything runs at 1.2GHz for 3.4us more).
  - A-half stores ride the otherwise-idle HWDGE rings; B-half stores go on
    SWDGE right after each drain. The final tile's B-half computes as two
    256-col PSUM groups so its first half's drain+store overlaps the last
    matmuls. out DRAM is per-partition packed; host unpermutes.
"""

import sys

if "/opt/trn_rl_repo" not in sys.path:
    sys.path.insert(0, "/opt/trn_rl_repo")

import numpy as np

B, S, E = 2, 4096, 1024
N_CORES = 8
ROWS = B * S            # 8192
R = ROWS // N_CORES     # 1024 rows per core
P = 128                 # partitions
KT = E // P             # 8 contraction tiles
MT = R // P             # 8 row tiles per core
NH = 2                  # n-half passes (512 output cols each)
NSZ = E // NH           # 512 = one PSUM bank (fp32)

# combined-input column offsets (e3m4 elements per partition)
#   [wvA k0-3 | x m0 | wvA k4-7 | x m1 | x m2 | x m3 | x m4 | x m5 | x m6
#    | x m7 | wvB k0-7]
XM = KT * P             # 1024 cols per x m-tile
OFF_WVA0 = 0            # wvA k0-3   (4*NSZ = 2048)
OFF_XM0 = 2048          # x m0       (1024)
OFF_WVA4 = 3072         # wvA k4-7   (2048)
OFF_X = 4096            # x m1..m7 at OFF_X + m*XM (m0 lives at OFF_XM0)
OFF_WVB = 12288         # wvB k0-7   (4096)
TOT = 16384


def _x_off(m):
    return OFF_XM0 if m == 0 else OFF_X + m * XM


def _wv_off(h, k):
    if h == 0:
        return k * NSZ if k < 4 else OFF_WVA4 + (k - 4) * NSZ
    return OFF_WVB + k * NSZ


_NC_CACHE = {}


def _build_nc(**bass_kwargs):
    import concourse.bass as bass
    import concourse.mybir as mybir
    from concourse import bacc
    from concourse.tile import TileContext

    f32 = mybir.dt.float32
    bf16 = mybir.dt.bfloat16
    fp8 = mybir.dt.float8e3
    nc = bacc.Bacc(None, target_bir_lowering=False, **bass_kwargs)

    inp = nc.dram_tensor("inp", [P, TOT], fp8, kind="ExternalInput")
    # per-partition packed output: outp[p, m*E + c] = out_row[m*P + p, c]
    out = nc.dram_tensor("out", [P, MT * E], bf16, kind="ExternalOutput")

    with TileContext(nc) as tc:
        with (
            tc.tile_pool(name="consts", bufs=1) as consts,
            tc.tile_pool(name="ipool", bufs=1) as ipool,
            tc.tile_pool(name="opool", bufs=MT) as opool,
            tc.tile_pool(name="ppool", bufs=7, space="PSUM") as ppool,
            tc.tile_pool(name="dpool", bufs=1, space="PSUM") as dpool,
        ):
            # PE warm-up bridge: preamble end (~7.9us) to head chunk (~11us).
            dum_sb = consts.tile([P, NSZ], bf16, tag="dum")
            nc.vector.memset(dum_sb, 0.0)
            dum_ps = dpool.tile([P, NSZ], f32, tag="dps")
            for _ in range(7):
                nc.tensor.matmul(
                    dum_ps, dum_sb[:, :P], dum_sb, start=True, stop=True
                )
            for _ in range(2):
                nc.tensor.matmul(
                    dum_ps[:, :P], dum_sb[:, :P], dum_sb[:, :P],
                    start=True, stop=True,
                )

            inp_sb = ipool.tile([P, TOT], fp8, tag="inp")

            def load(ring, c0, c1):
                ring.dma_start(out=inp_sb[:, c0:c1], in_=inp[:, c0:c1])

            load(nc.sync, 0, OFF_WVA4)            # wvA k0-3 + x m0   (384KB)
            load(nc.sync, OFF_WVA4, OFF_X + XM)   # wvA k4-7          (256KB)
            load(nc.scalar, OFF_X + XM, OFF_X + 3 * XM)   # x m1 m2   (256KB)
            load(nc.gpsimd, OFF_X + 3 * XM, OFF_X + 5 * XM)  # x m3 m4
            load(nc.gpsimd, OFF_X + 5 * XM, OFF_X + 7 * XM)  # x m5 m6
            load(nc.gpsimd, OFF_X + 7 * XM, TOT)  # x m7 + wvB        (640KB)

            om_tiles = [
                opool.tile([P, E], bf16, name=f"om{m}", tag="om")
                for m in range(MT)
            ]

            def store_cols(m, c0, c1, ring):
                dst = bass.AP(
                    tensor=out.tensor if hasattr(out, "tensor") else out,
                    offset=m * E + c0,
                    ap=[[MT * E, P], [1, c1 - c0]],
                )
                ring.dma_start(out=dst, in_=om_tiles[m][:, c0:c1])

            def drain(m, c0, c1, ps, ring):
                nc.vector.tensor_copy(out=om_tiles[m][:, c0:c1], in_=ps)
                store_cols(m, c0, c1, ring)

            def fillers(n):
                for _ in range(n):
                    nc.tensor.matmul(
                        dum_ps[:, :P], dum_sb[:, :P], dum_sb[:, :P],
                        start=True, stop=True,
                    )

            def mm(m, h, k, ps, pc0, pc1, start, stop):
                nc.tensor.matmul(
                    ps[:, pc0:pc1],
                    inp_sb[:, _x_off(m) + k * P : _x_off(m) + (k + 1) * P],
                    inp_sb[:, _wv_off(h, k) + pc0 : _wv_off(h, k) + pc1],
                    start=start,
                    stop=stop,
                )

            def mblock(m, h, ps):
                for k in range(KT):
                    mm(m, h, k, ps, 0, NSZ, k == 0, k == KT - 1)
                    if h == 0 and m == 0 and k == 3:
                        fillers(3)   # bridge wvA k4-7 (sync c2)

            # A-pass: m0..m7, then B-pass m0..m6 (B-halves stored on SWDGE),
            # then m7-B as two 256-col groups so drain+store overlaps the
            # final matmuls.
            for m in range(MT):
                ps = ppool.tile([P, NSZ], f32, name=f"psa{m}", tag="ps")
                mblock(m, 0, ps)
                drain(m, 0, NSZ, ps, nc.sync if (m % 2 == 0) else nc.scalar)
            for m in range(MT - 1):
                ps = ppool.tile([P, NSZ], f32, name=f"psb{m}", tag="ps")
                mblock(m, 1, ps)
                drain(m, NSZ, E, ps, nc.gpsimd)
            ps = ppool.tile([P, NSZ], f32, name="psb7", tag="ps")
            for g in range(2):
                pc0, pc1 = g * (NSZ // 2), (g + 1) * (NSZ // 2)
                for k in range(KT):
                    mm(MT - 1, 1, k, ps, pc0, pc1, k == 0, k == KT - 1)
                nc.vector.tensor_copy(
                    out=om_tiles[MT - 1][:, NSZ + pc0 : NSZ + pc1],
                    in_=ps[:, pc0:pc1],
                )
                store_cols(MT - 1, NSZ + pc0, NSZ + pc1, nc.gpsimd)
    nc.compile()
    return nc


def _get_nc():
    if "nc" not in _NC_CACHE:
        _NC_CACHE["nc"] = _build_nc()
    return _NC_CACHE["nc"]


def _prep_in_maps(x, Wv):
    import ml_dtypes

    e3m4 = ml_dtypes.float8_e3m4
    x = np.ascontiguousarray(np.asarray(x, dtype=np.float32))
    Wv = np.asarray(Wv, dtype=np.float32)

    xf = x.reshape(ROWS, E)
    # wvp[p, (h*KT + k)*NSZ + c] = 64*Wv[h*NSZ + c, k*P + p]
    wvp = (
        (Wv * 64.0)
        .reshape(NH, NSZ, KT, P)
        .transpose(3, 0, 2, 1)
        .reshape(P, NH * KT * NSZ)
        .astype(e3m4)
    )

    in_maps = []
    for c in range(N_CORES):
        xs = xf[c * R : (c + 1) * R]                    # [R, E]
        # xbc[p, (m*KT+k)*P+mm] = xs[m*P+mm, k*P+p]
        xbc = (
            xs.reshape(MT, P, KT, P)
            .transpose(3, 0, 2, 1)
            .reshape(P, MT * KT * P)
            .astype(e3m4)
        )
        inp = np.empty((P, TOT), dtype=e3m4)
        inp[:, OFF_WVA0:OFF_XM0] = wvp[:, 0 : 4 * NSZ]          # wvA k0-3
        inp[:, OFF_WVA4 : OFF_X + XM] = wvp[:, 4 * NSZ : 8 * NSZ]  # wvA k4-7
        inp[:, OFF_WVB:TOT] = wvp[:, 8 * NSZ : 16 * NSZ]        # wvB
        for m in range(MT):
            o = _x_off(m)
            inp[:, o : o + XM] = xbc[:, m * XM : (m + 1) * XM]
        in_maps.append({"inp": np.ascontiguousarray(inp)})
    return in_maps


def _install_ntff_hook():
    """This image's antenv lacks axon_hooks; recreate the bridge module so
    run_bass_kernel_spmd(trace=True) can reach the ctypes NTFF profiler."""
    import types

    if "antenv.axon_hooks" in sys.modules:
        return
    try:
        from trn_agent_boot.trn_boot import _ntff_profile_via_ctypes
    except ImportError:
        return
    hook = _ntff_profile_via_ctypes("/opt/axon/libaxon_pjrt.so")
    mod = types.ModuleType("antenv.axon_hooks")
    mod._hook = hook
    mod.get_axon_ntff_profile_hook = lambda: mod._hook
    mod.set_axon_ntff_profile_hook = lambda h: setattr(mod, "_hook", h)
    sys.modules["antenv.axon_hooks"] = mod


def _run(x, Wv, bv, trace=False):
    from concourse.bass_utils import run_bass_kernel_spmd

    if trace:
        _install_ntff_hook()
    nc = _get_nc()
    in_maps = _prep_in_maps(x, Wv)
    res = run_bass_kernel_spmd(
        nc, in_maps, core_ids=list(range(N_CORES)), trace=trace
    )
    # outp[p, m*E + c] = out_row[m*P + p, c]  ->  [R, E]
    shards = []
    for c in range(N_CORES):
        o = np.asarray(res.results[c]["out"])            # [P, MT*E]
        shards.append(
            o.reshape(P, MT, E).transpose(1, 0, 2).reshape(R, E)
        )
    out = np.concatenate(shards, axis=0)
    out = out.reshape(B, S, E).astype(np.float32) * (1.0 / 64.0)
    out += np.asarray(bv, dtype=np.float32)
    return out, res


def kernel(x, Wq, bq, Wk, bk, Wv, bv, weights):
    out, _ = _run(x, Wv, bv, trace=False)
    return out


def kernel_traced(x, Wq, bq, Wk, bk, Wv, bv, weights):
    """Like kernel() but with NTFF profiling; returns (out, BassKernelResults)."""
    out, res = _run(x, Wv, bv, trace=True)
    return out, res


# revision 13
# speedup vs baseline: 1.0902x; 1.0339x over previous
"""Trainium2 kernel for nn_KernelizedAttention_14869176779022.

Math note: the reference computes
    out = (s * v) / s        with s = <phi_q, phi_k> > 0  (sums of exps)
so out == v == x @ Wv.T + bv exactly (up to one multiply/divide rounding).
The kernel therefore only computes the Wv linear layer.

Sharding: data-parallel over the 8192 (B*S) positions - 1024 rows per core.
Wv (pre-transposed, x64-scaled) is replicated; the x64 unscale and the +bv
bias ride the host-side unshard pass (which already upcasts bf16 -> f32).

v5 design (measured-queue-model driven):
  - All inputs e3m4; PE consumes fp8 directly in normal mode (bf16 rate,
    numerically identical to a cast path since e3m4 -> bf16 is exact).
  - Inputs are packed host-side into ONE DRAM tensor in exact consumption
    order, so each DMA chunk is a contiguous block with 2-5KB partition
    lines (HWDGE rings crawl at ~40GB/s on 1KB lines vs ~125+ on 2KB+).
  - The sync ring (lowest latency, most predictable) carries the critical
    head: [wvA k0-3 | x m0] then [wvA k4-7]. Scalar carries [x m1 m2].
    SWDGE (whose chunk semaphores fire late while followers interleave)
    carries only slack-tolerant bulk: [x m3 m4], [x m5 m6], [x m7 | wvB].
  - Dummy matmuls bridge preamble-end (~7.9us) to the head chunk (~11us)
    with no idle hole (any multi-us hole resets the HAM clock-gate's 3.4us
    busy window and everything runs at 1.2GHz for 3.4us more).
  - A-half stores ride the otherwise-idle HWDGE rings; B-half stores go on
    SWDGE right after each drain. The final tile's B-half computes as two
    256-col PSUM groups so its first half's drain+store overlaps the last
    matmuls. out DRAM is per-partition packed; host unpermutes.
"""

import sys

if "/opt/trn_rl_repo" not in sys.path:
    sys.path.insert(0, "/opt/trn_rl_repo")

import numpy as np

B, S, E = 2, 4096, 1024
N_CORES = 8
ROWS = B * S            # 8192
R = ROWS // N_CORES     # 1024 rows per core
P = 128                 # partitions
KT = E // P             # 8 contraction tiles
MT = R // P             # 8 row tiles per core
NH = 2                  # n-half passes (512 output cols each)
NSZ = E // NH           # 512 = one PSUM bank (fp32)

# combined-input column offsets (e3m4 elements per partition). wv-A and x m0
# interleave by k-parity so BOTH HWDGE rings deliver the whole first m-block:
#   sync  c1 [0:2560):    [wvk0|xm0k0|wvk2|xm0k2|wvk4|xm0k4|wvk6|xm0k6]
#   scalar c1 [2560:5120): [wvk1|xm0k1|wvk3|xm0k3|wvk5|xm0k5|wvk7|xm0k7]
#   SWDGE:    [x m1 m2], [x m3 m4], [x m5 m6], [x m7 | wvB k0-7]
XM = KT * P             # 1024 cols per x m-tile
OFF_ODD = 2560
OFF_X = 4096            # x m1..m7 at OFF_X + m*XM (m0 is interleaved above)
OFF_WVB = 12288         # wvB k0-7   (4096)
TOT = 16384


def _wv_off(h, k):
    if h == 1:
        return OFF_WVB + k * NSZ
    return (k // 2) * (NSZ + P) + (OFF_ODD if (k % 2) else 0)


def _x_off(m, k):
    if m == 0:
        return _wv_off(0, k) + NSZ
    return OFF_X + m * XM + k * P


_NC_CACHE = {}


def _build_nc(**bass_kwargs):
    import concourse.bass as bass
    import concourse.mybir as mybir
    from concourse import bacc
    from concourse.tile import TileContext

    f32 = mybir.dt.float32
    bf16 = mybir.dt.bfloat16
    fp8 = mybir.dt.float8e3
    nc = bacc.Bacc(None, target_bir_lowering=False, **bass_kwargs)

    inp = nc.dram_tensor("inp", [P, TOT], fp8, kind="ExternalInput")
    # per-partition packed output: outp[p, m*E + c] = out_row[m*P + p, c]
    out = nc.dram_tensor("out", [P, MT * E], bf16, kind="ExternalOutput")

    with TileContext(nc) as tc:
        with (
            tc.tile_pool(name="consts", bufs=1) as consts,
            tc.tile_pool(name="ipool", bufs=1) as ipool,
            tc.tile_pool(name="opool", bufs=MT) as opool,
            tc.tile_pool(name="ppool", bufs=7, space="PSUM") as ppool,
            tc.tile_pool(name="dpool", bufs=1, space="PSUM") as dpool,
        ):
            # PE warm-up bridge: preamble end (~7.9us) to head chunk (~11us).
            dum_sb = consts.tile([P, NSZ], bf16, tag="dum")
            nc.vector.memset(dum_sb, 0.0)
            dum_ps = dpool.tile([P, NSZ], f32, tag="dps")
            for _ in range(7):
                nc.tensor.matmul(
                    dum_ps, dum_sb[:, :P], dum_sb, start=True, stop=True
                )
            for _ in range(2):
                nc.tensor.matmul(
                    dum_ps[:, :P], dum_sb[:, :P], dum_sb[:, :P],
                    start=True, stop=True,
                )

            inp_sb = ipool.tile([P, TOT], fp8, tag="inp")

            def load(ring, c0, c1):
                ring.dma_start(out=inp_sb[:, c0:c1], in_=inp[:, c0:c1])

            load(nc.sync, 0, OFF_ODD)             # wv even-k + xm0 even-k
            load(nc.scalar, OFF_ODD, OFF_X + XM)  # wv odd-k + xm0 odd-k
            load(nc.gpsimd, OFF_X + XM, OFF_X + 3 * XM)      # x m1 m2
            load(nc.gpsimd, OFF_X + 3 * XM, OFF_X + 5 * XM)  # x m3 m4
            load(nc.gpsimd, OFF_X + 5 * XM, OFF_X + 7 * XM)  # x m5 m6
            load(nc.gpsimd, OFF_X + 7 * XM, TOT)  # x m7 + wvB        (640KB)

            om_tiles = [
                opool.tile([P, E], bf16, name=f"om{m}", tag="om")
                for m in range(MT)
            ]

            def store_cols(m, c0, c1, ring):
                dst = bass.AP(
                    tensor=out.tensor if hasattr(out, "tensor") else out,
                    offset=m * E + c0,
                    ap=[[MT * E, P], [1, c1 - c0]],
                )
                ring.dma_start(out=dst, in_=om_tiles[m][:, c0:c1])

            def drain(m, c0, c1, ps, ring):
                nc.vector.tensor_copy(out=om_tiles[m][:, c0:c1], in_=ps)
                store_cols(m, c0, c1, ring)

            def fillers(n):
                for _ in range(n):
                    nc.tensor.matmul(
                        dum_ps[:, :P], dum_sb[:, :P], dum_sb[:, :P],
                        start=True, stop=True,
                    )

            def mm(m, h, k, ps, wc0, wc1, start, stop):
                nc.tensor.matmul(
                    ps,
                    inp_sb[:, _x_off(m, k) : _x_off(m, k) + P],
                    inp_sb[:, _wv_off(h, k) + wc0 : _wv_off(h, k) + wc1],
                    start=start,
                    stop=stop,
                )

            # m0 consumes even-k (sync, lands first) before odd-k (scalar).
            M0_KS = [0, 2, 4, 6, 1, 3, 5, 7]

            def mblock(m, h, ps):
                ks = M0_KS if (h == 0 and m == 0) else range(KT)
                for i, k in enumerate(ks):
                    mm(m, h, k, ps, 0, NSZ, i == 0, i == KT - 1)
                    if h == 0 and m == 0 and i == 3:
                        fillers(2)   # bridge scalar c1 (odd-k) arrival

            # A-pass: m0..m7, then B-pass m0..m6 (B-halves stored on SWDGE),
            # then m7-B as two 256-col groups (separate PSUM tiles, so the
            # first group's drain+store overlaps the final matmuls).
            for m in range(MT):
                ps = ppool.tile([P, NSZ], f32, name=f"psa{m}", tag="ps")
                mblock(m, 0, ps)
                drain(m, 0, NSZ, ps, nc.sync if (m % 2 == 0) else nc.scalar)
            for m in range(MT - 1):
                ps = ppool.tile([P, NSZ], f32, name=f"psb{m}", tag="ps")
                mblock(m, 1, ps)
                drain(m, NSZ, E, ps, nc.gpsimd)
            for g in range(2):
                pc0, pc1 = g * (NSZ // 2), (g + 1) * (NSZ // 2)
                psg = ppool.tile([P, NSZ // 2], f32, name=f"psb7{g}", tag="ps")
                for k in range(KT):
                    mm(MT - 1, 1, k, psg, pc0, pc1, k == 0, k == KT - 1)
                nc.vector.tensor_copy(
                    out=om_tiles[MT - 1][:, NSZ + pc0 : NSZ + pc1], in_=psg
                )
                store_cols(MT - 1, NSZ + pc0, NSZ + pc1, nc.gpsimd)
    nc.compile()
    return nc


def _get_nc():
    if "nc" not in _NC_CACHE:
        _NC_CACHE["nc"] = _build_nc()
    return _NC_CACHE["nc"]


def _prep_in_maps(x, Wv):
    import ml_dtypes

    e3m4 = ml_dtypes.float8_e3m4
    x = np.ascontiguousarray(np.asarray(x, dtype=np.float32))
    Wv = np.asarray(Wv, dtype=np.float32)

    xf = x.reshape(ROWS, E)
    # wvp[p, (h*KT + k)*NSZ + c] = 64*Wv[h*NSZ + c, k*P + p]
    wvp = (
        (Wv * 64.0)
        .reshape(NH, NSZ, KT, P)
        .transpose(3, 0, 2, 1)
        .reshape(P, NH * KT * NSZ)
        .astype(e3m4)
    )

    in_maps = []
    for c in range(N_CORES):
        xs = xf[c * R : (c + 1) * R]                    # [R, E]
        # xbc[p, (m*KT+k)*P+mm] = xs[m*P+mm, k*P+p]
        xbc = (
            xs.reshape(MT, P, KT, P)
            .transpose(3, 0, 2, 1)
            .reshape(P, MT * KT * P)
            .astype(e3m4)
        )
        inp = np.empty((P, TOT), dtype=e3m4)
        for k in range(KT):
            o = _wv_off(0, k)
            inp[:, o : o + NSZ] = wvp[:, k * NSZ : (k + 1) * NSZ]
            inp[:, o + NSZ : o + NSZ + P] = xbc[:, k * P : (k + 1) * P]
        inp[:, OFF_WVB:TOT] = wvp[:, 8 * NSZ : 16 * NSZ]        # wvB
        for m in range(1, MT):
            o = OFF_X + m * XM
            inp[:, o : o + XM] = xbc[:, m * XM : (m + 1) * XM]
        in_maps.append({"inp": np.ascontiguousarray(inp)})
    return in_maps


def _install_ntff_hook():
    """This image's antenv lacks axon_hooks; recreate the bridge module so
    run_bass_kernel_spmd(trace=True) can reach the ctypes NTFF profiler."""
    import types

    if "antenv.axon_hooks" in sys.modules:
        return
    try:
        from trn_agent_boot.trn_boot import _ntff_profile_via_ctypes
    except ImportError:
        return
    hook = _ntff_profile_via_ctypes("/opt/axon/libaxon_pjrt.so")
    mod = types.ModuleType("antenv.axon_hooks")
    mod._hook = hook
    mod.get_axon_ntff_profile_hook = lambda: mod._hook
    mod.set_axon_ntff_profile_hook = lambda h: setattr(mod, "_hook", h)
    sys.modules["antenv.axon_hooks"] = mod


def _run(x, Wv, bv, trace=False):
    from concourse.bass_utils import run_bass_kernel_spmd

    if trace:
        _install_ntff_hook()
    nc = _get_nc()
    in_maps = _prep_in_maps(x, Wv)
    res = run_bass_kernel_spmd(
        nc, in_maps, core_ids=list(range(N_CORES)), trace=trace
    )
    # outp[p, m*E + c] = out_row[m*P + p, c]  ->  [R, E]
    shards = []
    for c in range(N_CORES):
        o = np.asarray(res.results[c]["out"])            # [P, MT*E]
        shards.append(
            o.reshape(P, MT, E).transpose(1, 0, 2).reshape(R, E)
        )
    out = np.concatenate(shards, axis=0)
    out = out.reshape(B, S, E).astype(np.float32) * (1.0 / 64.0)
    out += np.asarray(bv, dtype=np.float32)
    return out, res


def kernel(x, Wq, bq, Wk, bk, Wv, bv, weights):
    out, _ = _run(x, Wv, bv, trace=False)
    return out


def kernel_traced(x, Wq, bq, Wk, bk, Wv, bv, weights):
    """Like kernel() but with NTFF profiling; returns (out, BassKernelResults)."""
    out, res = _run(x, Wv, bv, trace=True)
    return out, res
